# revision 1
# baseline (speedup 1.0000x reference)
"""Trainium2 Bass kernel for nn_BertAttentionDistance (B=4, S=2048, H=1024, NH=1, DT=32).

Sharding: 8 cores = (batch b = c//2) x (query-half qh = c%2, 1024 q-rows each).
K/V projection work for a batch is replicated across its 2 cores (no collectives).

Math notes (exact identities w.r.t. the reference):
  * take_along_axis(word_dot_distance, rel, 3) * (rel == 1)
      == (q . dist_emb[1]) * (rel == 1)           (gather collapses)
  * softmax max-subtraction skipped: scores/32 is O(+-3), safely in fp32 exp range.
  * attention_mask enters as per-k additive bias inside exp():
      exp((s + dist)/32 + am_k)  -- the ACT per-partition bias slot (k on partitions).
  * v-bias and o-bias fold into the residual: x = Wo@ctx + (Wo@bv + bo) + hs,
      folded on host into hsq' = hs_q + Wo@bv + bo.
  * q/k biases applied on the PSUM->SBUF evacuation (per-partition tensor_scalar add).
  * softmax 1/denominator folded into the ctxT PSUM->SBUF evacuation multiply.

Dataflow (per core, all matmul inputs bf16, fp32 PSUM accumulate):
  v[s,d]   = hsT^T-chunks @ WvT         (lhsT=hsT chunk, rhs=WvT)   [s-part, d-free]
  kT[h,k]  = (Wk^T)^T-chunks @ hsT      (lhsT=WkT chunk, rhs=hsT)   [d-part, k-free]
  qT[h,q]  likewise from hsqT (this core's query rows)
  ST[k,q]  = kT^T @ qT                  (lhsT=kT chunk,  rhs=qT)    transposed scores
  expT     = exp((ST + maskT*wdd1)/32 + am_k)    maskT = (relT == 1)
  denom[q] = ones^T @ expT (PE row-sum) -> gpsimd partition_broadcast -> 1/denom
  ctxT[d,q]= (v^T-chunks @ expT) * (1/denom)[q]  (lhsT=v chunk, rhs=expT)
  attn[s,h]= ctxT^T-chunks @ WoT        (lhsT=ctxT chunk, rhs=WoT)
  x        = attn + hsq'                then LayerNorm over h.
The epilogue (out-proj + LN) for query-chunk 0 is emitted right after its PV so
it overlaps the PE shadow of query-chunk 1's scores/PV.
"""

import sys

for p in ("/opt/trn_rl_repo", "/opt/pypackages"):
    if p not in sys.path:
        sys.path.insert(0, p)

from contextlib import ExitStack

import ml_dtypes
import numpy as np

import concourse.bacc as bacc
import concourse.bass as bass
import concourse.tile as tile
from concourse import mybir
from concourse.bass_utils import run_bass_kernel_spmd

# Problem constants (hardcoded per the harness contract).
B, S, H = 4, 2048, 1024
N_CORES = 8
SQ = 1024          # query rows per core
HC = H // 128      # 8 chunks of 128 over hidden/head dim
KC = S // 128      # 16 chunks of 128 over key dim
QN = SQ // 512     # 2 chunks of 512 over this core's query rows
LN_EPS = 1e-12
INV_SQRT_DH = 1.0 / 32.0

F32 = mybir.dt.float32
BF16 = mybir.dt.bfloat16
I8 = mybir.dt.int8
F8 = mybir.dt.float8e4
DR = mybir.MatmulPerfMode.DoubleRow

_CACHE = {}


def _build_program(ln_affine=True):
    nc = bacc.Bacc("TRN2", target_bir_lowering=False, debug=False)

    hsT = nc.dram_tensor("hsT", [H, S], F8, kind="ExternalInput")
    hsqT = nc.dram_tensor("hsqT", [H, SQ], F8, kind="ExternalInput")
    hsq = nc.dram_tensor("hsq", [SQ, H], F32, kind="ExternalInput")
    relT = nc.dram_tensor("relT", [S, SQ], F8, kind="ExternalInput")
    wqT = nc.dram_tensor("wqT", [H, H], F8, kind="ExternalInput")
    wkT = nc.dram_tensor("wkT", [H, H], F8, kind="ExternalInput")
    wvT = nc.dram_tensor("wvT", [H, H], F8, kind="ExternalInput")
    woT = nc.dram_tensor("woT", [H, H], BF16, kind="ExternalInput")
    bq_d = nc.dram_tensor("bq", [128, HC], F32, kind="ExternalInput")
    bk_d = nc.dram_tensor("bk", [128, HC], F32, kind="ExternalInput")
    d1_d = nc.dram_tensor("d1", [128, HC], F8, kind="ExternalInput")
    am_d = nc.dram_tensor("am", [128, KC], F32, kind="ExternalInput")
    lng_d = nc.dram_tensor("lng", [H], F32, kind="ExternalInput")
    lnb_d = nc.dram_tensor("lnb", [H], F32, kind="ExternalInput")
    out_d = nc.dram_tensor("out", [SQ, H], F32, kind="ExternalOutput")

    def bcast_rows(src_1d_ap, p=128):
        """AP that reads a 1-D DRAM row broadcast across p partitions."""
        return bass.AP(
            tensor=src_1d_ap.tensor,
            offset=src_1d_ap.offset,
            ap=[[0, p], *src_1d_ap.ap],
        )

    with tile.TileContext(nc) as tc, ExitStack() as ctx:
        consts = ctx.enter_context(tc.tile_pool(name="consts", bufs=1))
        big = ctx.enter_context(tc.tile_pool(name="big", bufs=1))
        psum_sm = ctx.enter_context(tc.tile_pool(name="psum_sm", bufs=1, space="PSUM"))

        # ---- constants ----
        ones_t = consts.tile([128, 1], F8)
        nc.vector.memset(ones_t, 1.0)
        eps_t = consts.tile([128, 1], F32)
        nc.vector.memset(eps_t, LN_EPS)
        bq_t = consts.tile([128, HC], F32)
        nc.sync.dma_start(bq_t, bq_d[:])
        bk_t = consts.tile([128, HC], F32)
        nc.sync.dma_start(bk_t, bk_d[:])
        d1_t = consts.tile([128, HC], F8)
        nc.sync.dma_start(d1_t, d1_d[:])
        am_t = consts.tile([128, KC], F32)
        nc.sync.dma_start(am_t, am_d[:])
        g_b = consts.tile([128, H], F32)
        nc.gpsimd.dma_start(g_b, bcast_rows(lng_d[:]))
        b_b = consts.tile([128, H], F32)
        nc.gpsimd.dma_start(b_b, bcast_rows(lnb_d[:]))

        # ---- persistent big tensors ----
        kT = big.tile([128, HC, S], F8)       # [d-part, dc, k]
        v_sb = big.tile([128, KC, H], F8)     # [s-part, kc, d]
        ctxT = big.tile([128, HC, SQ], BF16)    # [d-part, dc, q] (normalized)

        with tc.tile_pool(name="qpool", bufs=1) as qpool:
            qT = qpool.tile([128, HC, SQ], F8)    # [d-part, dc, q]
            wdd1_b = qpool.tile([128, SQ], F32)     # broadcast of q . dist_emb[1]
            wdd1_row = qpool.tile([1, SQ], F32)

            # ================= phase 1: projections =================
            with (
                tc.tile_pool(name="hsT_pool", bufs=1) as hsT_pool,
                tc.tile_pool(name="wpool", bufs=2) as wpool,
                tc.tile_pool(name="psum_p", bufs=4, space="PSUM") as psum_p,
            ):
                # DMA order paced to v-projection consumption: wv half 0 +
                # hsT column block 0 first (first matmul after ~2MB), the
                # rest interleaved.
                wv_sb = wpool.tile([128, HC, H], F8, tag="w")
                wv_r = wvT[:].rearrange("(c p) m -> p c m", p=128)
                hsT_sb = hsT_pool.tile([128, HC, S], F8)
                hsT_r = hsT[:].rearrange("(c p) s -> p c s", p=128)

                def dma_hsT_block(nb):
                    nc.sync.dma_start(
                        hsT_sb[:, :, nb * 512:(nb + 1) * 512],
                        hsT_r[:, :, nb * 512:(nb + 1) * 512],
                    )

                nc.sync.dma_start(wv_sb[:, :, 0:512], wv_r[:, :, 0:512])
                dma_hsT_block(0)
                dma_hsT_block(1)
                nc.sync.dma_start(wv_sb[:, :, 512:1024], wv_r[:, :, 512:1024])
                dma_hsT_block(2)
                dma_hsT_block(3)

                # v[s,d]: lhsT = hsT chunk (stationary), rhs = WvT columns
                for n0 in range(0, H, 512):
                    for sc in range(KC):
                        ps = psum_p.tile([128, 512], F32, tag="pp")
                        for dc in range(0, HC, 2):
                            nc.tensor.matmul(
                                ps,
                                hsT_sb[:, dc:dc + 2, sc * 128:(sc + 1) * 128],
                                wv_sb[:, dc:dc + 2, n0:n0 + 512],
                                start=(dc == 0),
                                stop=(dc == HC - 2),
                                perf_mode=DR,
                            )
                        nc.any.tensor_scalar_mul(v_sb[:, sc, n0:n0 + 512], ps, 0.125)

                # k then q projections: dst[h=mc, col n] += W^T-chunk @ src
                for w_d, bias_t, dst, src, ncols in (
                    (wkT, bk_t, kT, hsT_sb, S),
                    (wqT, bq_t, qT, None, SQ),
                ):
                    w_sb = wpool.tile([128, HC, H], F8, tag="w")
                    w_r = w_d[:].rearrange("(c p) m -> p c m", p=128)
                    for mc in range(HC):
                        nc.sync.dma_start(
                            w_sb[:, :, mc * 128:(mc + 1) * 128],
                            w_r[:, :, mc * 128:(mc + 1) * 128],
                        )
                    if src is None:  # q: DMA this core's query rows
                        src = hsT_pool.tile([128, HC, SQ], F8)
                        hsqT_r = hsqT[:].rearrange("(c p) s -> p c s", p=128)
                        for nb in range(2):
                            nc.sync.dma_start(
                                src[:, :, nb * 512:(nb + 1) * 512],
                                hsqT_r[:, :, nb * 512:(nb + 1) * 512],
                            )
                    for n0 in range(0, ncols, 512):
                        for mc in range(HC):
                            ps = psum_p.tile([128, 512], F32, tag="pp")
                            for dc in range(0, HC, 2):
                                nc.tensor.matmul(
                                    ps,
                                    w_sb[:, dc:dc + 2, mc * 128:(mc + 1) * 128],
                                    src[:, dc:dc + 2, n0:n0 + 512],
                                    start=(dc == 0),
                                    stop=(dc == HC - 2),
                                    perf_mode=DR,
                                )
                            nc.any.tensor_scalar(
                                dst[:, mc, n0:n0 + 512], ps,
                                0.125, bias_t[:, mc:mc + 1],
                                mybir.AluOpType.mult, mybir.AluOpType.add,
                            )

                # wdd1[q] = q . dist_emb[1] (M=1 matmuls), partition-broadcast
                for qn in range(QN):
                    q_sl = slice(qn * 512, (qn + 1) * 512)
                    ps1 = psum_sm.tile([1, 512], F32, tag="small")
                    for dc in range(HC):
                        nc.tensor.matmul(
                            ps1,
                            d1_t[:, dc:dc + 1],
                            qT[:, dc, q_sl],
                            start=(dc == 0),
                            stop=(dc == HC - 1),
                        )
                    nc.any.tensor_scalar_mul(wdd1_row[:, q_sl], ps1, 0.125)
                nc.gpsimd.partition_broadcast(wdd1_b, wdd1_row)

            # ====== phase 2+3 interleaved per query-chunk of 512 ======
            with (
                tc.tile_pool(name="expp", bufs=2) as expp,
                tc.tile_pool(name="relp", bufs=2) as relp,
                tc.tile_pool(name="smx", bufs=3) as smx,
                tc.tile_pool(name="wo_pool", bufs=1) as wo_pool,
                tc.tile_pool(name="epi", bufs=3) as epi,
                tc.tile_pool(name="denp", bufs=2) as denp,
                tc.tile_pool(name="stat", bufs=4) as stat,
                tc.tile_pool(name="psum_s", bufs=4, space="PSUM") as psum_s,
                tc.tile_pool(name="psum_v", bufs=3, space="PSUM") as psum_v,
            ):
                wo_sb = wo_pool.tile([128, HC, H], BF16)
                wo_r = woT[:].rearrange("(c p) m -> p c m", p=128)
                for dc in range(HC):
                    nc.sync.dma_start(wo_sb[:, dc], wo_r[:, dc])

                def scores_phase(qn):
                    # Emits scores matmuls + softmax numerators AND the
                    # denominator row-sum matmuls, lagged one kc so the PE
                    # never waits on the exp chain.
                    q_sl = slice(qn * 512, (qn + 1) * 512)
                    expT = expp.tile([128, KC, 512], F8, tag="expT")
                    dn = psum_sm.tile([1, 512], F32, tag="small")
                    for kc in range(KC):
                        ps = psum_s.tile([128, 512], F32, tag="ps")
                        for dc in range(0, HC, 2):
                            nc.tensor.matmul(
                                ps,
                                kT[:, dc:dc + 2, kc * 128:(kc + 1) * 128],
                                qT[:, dc:dc + 2, q_sl],
                                start=(dc == 0),
                                stop=(dc == HC - 2),
                                perf_mode=DR,
                            )
                        rel_t = relp.tile([128, 512], F8, tag="rel")
                        nc.sync.dma_start(
                            rel_t, relT[kc * 128:(kc + 1) * 128, q_sl]
                        )
                        y_t = smx.tile([128, 512], F32, tag="y")
                        nc.any.tensor_mul(y_t, rel_t, wdd1_b[:, q_sl])
                        nc.any.tensor_add(y_t, ps, y_t)
                        # exp((qk + dist)/32 + attention_mask[k])
                        nc.scalar.activation(
                            expT[:, kc, :],
                            y_t,
                            mybir.ActivationFunctionType.Exp,
                            bias=am_t[:, kc:kc + 1],
                            scale=INV_SQRT_DH,
                        )
                    for kc in range(KC):
                        nc.tensor.matmul(
                            dn,
                            ones_t,
                            expT[:, kc, :],
                            start=(kc == 0),
                            stop=(kc == KC - 1),
                        )
                    dr = denp.tile([1, 512], F32, tag="dr")
                    nc.any.tensor_copy(dr, dn)
                    db = denp.tile([128, 512], F32, tag="db")
                    nc.gpsimd.partition_broadcast(db, dr)
                    rb = denp.tile([128, 512], F32, tag="rb")
                    nc.vector.reciprocal(rb, db)
                    return expT, rb

                def pv_phase(qn, expT, rb):
                    # PV: ctxT[d, q], normalized on evacuation
                    q_sl = slice(qn * 512, (qn + 1) * 512)
                    for dc in range(HC):
                        pv = psum_v.tile([128, 512], F32, tag="pv")
                        for kc in range(0, KC, 2):
                            nc.tensor.matmul(
                                pv,
                                v_sb[:, kc:kc + 2, dc * 128:(dc + 1) * 128],
                                expT[:, kc:kc + 2, :],
                                start=(kc == 0),
                                stop=(kc == KC - 2),
                                perf_mode=DR,
                            )
                        nc.any.tensor_mul(ctxT[:, dc, q_sl], pv, rb)

                def epilogue(qn):
                    # out-proj + residual + LN for this q-chunk
                    for sc in range(qn * 4, qn * 4 + 4):
                        x_t = epi.tile([128, H], F32, tag="x")
                        for hn in range(2):
                            ao = psum_v.tile([128, 512], F32, tag="pv")
                            for dc in range(HC):
                                nc.tensor.matmul(
                                    ao,
                                    ctxT[:, dc, sc * 128:(sc + 1) * 128],
                                    wo_sb[:, dc, hn * 512:(hn + 1) * 512],
                                    start=(dc == 0),
                                    stop=(dc == HC - 1),
                                )
                            h_sl = slice(hn * 512, (hn + 1) * 512)
                            # evacuate on ACT (has slack during the epilogue)
                            nc.scalar.activation(
                                x_t[:, h_sl], ao,
                                mybir.ActivationFunctionType.Copy,
                            )
                            # residual add via accumulating DMA (frees DVE)
                            nc.gpsimd.dma_start(
                                x_t[:, h_sl],
                                hsq[sc * 128:(sc + 1) * 128, h_sl],
                                accum_op=mybir.AluOpType.add,
                            )

                        # LayerNorm over h (free dim)
                        st = stat.tile([128, 2, 6], F32, tag="st")
                        nc.vector.bn_stats(st[:, 0, :], x_t[:, 0:512])
                        nc.vector.bn_stats(st[:, 1, :], x_t[:, 512:1024])
                        mv = stat.tile([128, 2], F32, tag="mv")
                        nc.vector.bn_aggr(mv, st)
                        sd = stat.tile([128, 1], F32, tag="sd")
                        nc.scalar.activation(
                            sd, mv[:, 1:2],
                            mybir.ActivationFunctionType.Sqrt, bias=eps_t,
                        )
                        rq = stat.tile([128, 1], F32, tag="rq")
                        nc.vector.reciprocal(rq, sd)
                        y_t = epi.tile([128, H], F32, tag="hsq")
                        nc.vector.tensor_scalar(
                            y_t, x_t, mv[:, 0:1], rq,
                            mybir.AluOpType.subtract, mybir.AluOpType.mult,
                        )
                        if ln_affine:
                            nc.any.tensor_mul(y_t, y_t, g_b)
                            nc.any.tensor_add(y_t, y_t, b_b)
                        nc.sync.dma_start(out_d[sc * 128:(sc + 1) * 128, :], y_t)

                # interleave: epilogue(0) fills the PE shadow between
                # scores(1) and PV(1); den/exp chains hide under matmuls.
                exp0, rb0 = scores_phase(0)
                pv_phase(0, exp0, rb0)
                exp1, rb1 = scores_phase(1)
                epilogue(0)
                pv_phase(1, exp1, rb1)
                epilogue(1)

    nc.compile()
    return nc


def get_program(ln_affine=True):
    key = ("nc", ln_affine)
    if key not in _CACHE:
        _CACHE[key] = _build_program(ln_affine)
    return _CACHE[key]


def make_in_maps(inputs):
    """Host-side sharding / layout prep (numpy only)."""
    f32 = np.float32
    bf16 = ml_dtypes.bfloat16
    hs = np.asarray(inputs["hidden_states"], dtype=f32)
    rel = np.asarray(inputs["word_word_relation"])
    am = np.asarray(inputs["attention_mask"], dtype=f32)  # [B,1,1,S]
    Wq = np.asarray(inputs["Wq"], dtype=f32)
    Wk = np.asarray(inputs["Wk"], dtype=f32)
    Wv = np.asarray(inputs["Wv"], dtype=f32)
    Wo = np.asarray(inputs["Wo"], dtype=f32)
    bq = np.asarray(inputs["bq"], dtype=f32)
    bk = np.asarray(inputs["bk"], dtype=f32)
    bv = np.asarray(inputs["bv"], dtype=f32)
    bo = np.asarray(inputs["bo"], dtype=f32)
    d1 = np.asarray(inputs["dist_emb"], dtype=f32)[1]
    lng = np.asarray(inputs["ln_g"], dtype=f32)
    lnb = np.asarray(inputs["ln_b"], dtype=f32)

    f8 = ml_dtypes.float8_e4m3
    wqT = np.ascontiguousarray(Wq.T * 8.0).astype(f8)
    wkT = np.ascontiguousarray(Wk.T * 8.0).astype(f8)
    wvT = np.ascontiguousarray(Wv.T * 8.0).astype(f8)
    woT = np.ascontiguousarray(Wo.T).astype(bf16)
    bo_eff = Wo @ bv + bo  # v/o biases fold into the residual
    bq_t = np.ascontiguousarray(bq.reshape(HC, 128).T)
    bk_t = np.ascontiguousarray(bk.reshape(HC, 128).T)
    d1_t = np.ascontiguousarray(d1.reshape(HC, 128).T * 8.0).astype(f8)
    rel8 = rel.astype(np.int8)

    in_maps = []
    for c in range(N_CORES):
        b, qh = divmod(c, 2)
        qs = qh * SQ
        in_maps.append({
            "hsT": hs[b].T.astype(f8),
            "hsqT": hs[b, qs:qs + SQ, :].T.astype(f8),
            "hsq": hs[b, qs:qs + SQ, :] + bo_eff,
            "relT": np.ascontiguousarray((rel8[b, qs:qs + SQ, :].T == 1)).astype(f8),
            "wqT": wqT, "wkT": wkT, "wvT": wvT, "woT": woT,
            "bq": bq_t, "bk": bk_t, "d1": d1_t,
            "am": np.ascontiguousarray(am[b, 0, 0].reshape(KC, 128).T),
            "lng": lng, "lnb": lnb,
        })
    return in_maps


def kernel(**inputs):
    ln_affine = not (
        np.all(np.asarray(inputs["ln_g"]) == 1.0)
        and np.all(np.asarray(inputs["ln_b"]) == 0.0)
    )
    nc = get_program(ln_affine)
    in_maps = make_in_maps(inputs)
    res = run_bass_kernel_spmd(nc, in_maps, core_ids=list(range(N_CORES)))
    out = np.empty((B, S, H), dtype=np.float32)
    for c in range(N_CORES):
        b, qh = divmod(c, 2)
        out[b, qh * SQ:(qh + 1) * SQ, :] = res.results[c]["out"]
    return out



# revision 6
# speedup vs baseline: 1.0641x; 1.0641x over previous
"""Trainium2 Bass kernel for nn_BertAttentionDistance (B=4, S=2048, H=1024, NH=1, DT=32).

Sharding: 8 cores = (batch b = c//2) x (query-half qh = c%2, 1024 q-rows each).
K/V projection work for a batch is replicated across its 2 cores (no collectives).
Key order per core is [own 1024 keys, other 1024 keys]; relT/am are permuted to
match on the host (softmax/PV are order-invariant over keys).

Math notes (exact identities w.r.t. the reference):
  * take_along_axis(word_dot_distance, rel, 3) * (rel == 1)
      == (q . dist_emb[1]) * (rel == 1)           (gather collapses)
  * wdd1 = q . d1 = hs . (Wq^T d1) + bq . d1 = hs . u + cq  (u, cq on host)
  * softmax max-subtraction skipped: scores/32 is O(+-3), safely in fp32 exp range.
  * v-bias and o-bias fold into the residual: x = Wo@ctx + (Wo@bv + bo) + hs,
      folded on host into hsq' = hs_q + Wo@bv + bo.
  * attention_mask folds into md = rel*wdd1/32 + am_k (general variant only).
  * softmax 1/denominator (x32 for the fp8 ctxT scale) folded into the ctxT
      PSUM->SBUF evacuation multiply.

Dataflow (per core, all matmul inputs fp8 + DoubleRow, fp32 PSUM):
  wdd1[q]  = u^T-chunks @ hsqT          (M=1 DR)  -> row, broadcast
  v[s,d]   = hs-half^T-chunks @ WvT     [s-part (own,other), d-free]
  kT[h,k]  = (Wk^T)^T-chunks @ hs-half  [d-part, k-free (own,other)]
  qT[h,q]  from hsqT
  md[k,q]  = relT * (wdd1/32)  bf16     (DVE, interleaved with projections)
  ST[k,q]  = kT^T @ qT; y = (ST/32 + md) on DVE (fused stt); expT = exp(y) on ACT
  denom[q] = ones^T @ expT (M=1 DR) -> *1/32 -> bcast -> 1/d  (rb32 = 32/denom)
  ctxT[d,q]= (v^T-chunks @ expT) * rb32   -> fp8 (holds 32*ctx)
  attn     = ctxT^T-chunks @ WoT (fp8 DR); x = attn/256 + hsq  (gpsimd stt)
  LayerNorm via bn_stats/aggr, ACT sqrt, DVE recip, ACT identity normalize.
"""

import sys

for p in ("/opt/trn_rl_repo", "/opt/pypackages"):
    if p not in sys.path:
        sys.path.insert(0, p)

from contextlib import ExitStack

import ml_dtypes
import numpy as np

import concourse.bacc as bacc
import concourse.bass as bass
import concourse.tile as tile
from concourse import mybir
from concourse.bass_utils import run_bass_kernel_spmd

# Problem constants (hardcoded per the harness contract).
B, S, H = 4, 2048, 1024
N_CORES = 8
SQ = 1024          # query rows per core
HC = H // 128      # 8 chunks of 128 over hidden/head dim
KC = S // 128      # 16 chunks of 128 over key dim
KCH = KC // 2      # key chunks per half
LN_EPS = 1e-12
INV_SQRT_DH = 1.0 / 32.0

F32 = mybir.dt.float32
BF16 = mybir.dt.bfloat16
F8 = mybir.dt.float8e4
DR = mybir.MatmulPerfMode.DoubleRow
MULT = mybir.AluOpType.mult
ADD = mybir.AluOpType.add

_CACHE = {}


def _build_program(am_zero=True, ln_affine=False):
    nc = bacc.Bacc("TRN2", target_bir_lowering=False, debug=False)

    hsqT = nc.dram_tensor("hsqT", [H, SQ], F8, kind="ExternalInput")
    hsoT = nc.dram_tensor("hsoT", [H, SQ], F8, kind="ExternalInput")
    hsq = nc.dram_tensor("hsq", [SQ, H], F32, kind="ExternalInput")
    relT = nc.dram_tensor("relT", [S, SQ], F8, kind="ExternalInput")
    wqT = nc.dram_tensor("wqT", [H, H], F8, kind="ExternalInput")
    wkT = nc.dram_tensor("wkT", [H, H], F8, kind="ExternalInput")
    wvT = nc.dram_tensor("wvT", [H, H], F8, kind="ExternalInput")
    woT = nc.dram_tensor("woT", [H, H], F8, kind="ExternalInput")
    u_d = nc.dram_tensor("u", [128, HC, 1], F8, kind="ExternalInput")
    cq_d = nc.dram_tensor("cq32", [1, 1], F32, kind="ExternalInput")
    bq_d = nc.dram_tensor("bq", [128, HC], F32, kind="ExternalInput")
    bk_d = nc.dram_tensor("bk", [128, HC], F32, kind="ExternalInput")
    if not am_zero:
        am_d = nc.dram_tensor("am", [128, KC], F32, kind="ExternalInput")
    if ln_affine:
        lng_d = nc.dram_tensor("lng", [H], F32, kind="ExternalInput")
        lnb_d = nc.dram_tensor("lnb", [H], F32, kind="ExternalInput")
    out_d = nc.dram_tensor("out", [SQ, H], F32, kind="ExternalOutput")

    def bcast_rows(src_1d_ap, p=128):
        return bass.AP(
            tensor=src_1d_ap.tensor,
            offset=src_1d_ap.offset,
            ap=[[0, p], *src_1d_ap.ap],
        )

    with tile.TileContext(nc) as tc, ExitStack() as ctx:
        consts = ctx.enter_context(tc.tile_pool(name="consts", bufs=1))
        big = ctx.enter_context(tc.tile_pool(name="big", bufs=1))
        psum_sm = ctx.enter_context(tc.tile_pool(name="psum_sm", bufs=2, space="PSUM"))

        # ---- constants (small, issued on gpsimd queue to keep sync free) ----
        # DR ldweights needs >=64B stride between the two k-tiles: pad dim2.
        ones2_pad = consts.tile([128, 2, 64], F8)
        nc.vector.memset(ones2_pad, 1.0)
        ones2 = ones2_pad[:, :, 0:1]
        eps_t = consts.tile([128, 1], F32)
        nc.vector.memset(eps_t, LN_EPS)
        u_pad = consts.tile([128, HC, 64], F8)
        u_t = u_pad[:, :, 0:1]
        nc.gpsimd.dma_start(u_t, u_d[:])
        cq_t = consts.tile([1, 1], F32)
        nc.gpsimd.dma_start(cq_t, cq_d[:])
        bq_t = consts.tile([128, HC], F32)
        nc.gpsimd.dma_start(bq_t, bq_d[:])
        bk_t = consts.tile([128, HC], F32)
        nc.gpsimd.dma_start(bk_t, bk_d[:])
        if not am_zero:
            am_t = consts.tile([128, KC], F32)
            nc.gpsimd.dma_start(am_t, am_d[:])
        if ln_affine:
            g_b = consts.tile([128, H], F32)
            nc.gpsimd.dma_start(g_b, bcast_rows(lng_d[:]))
            b_b = consts.tile([128, H], F32)
            nc.gpsimd.dma_start(b_b, bcast_rows(lnb_d[:]))

        # ---- persistent big tensors ----
        kT = big.tile([128, HC, S], F8)        # [d-part, dc, k(own,other)]
        v_sb = big.tile([128, KC, H], F8)      # [s-part(own,other), kc, d]
        qT = big.tile([128, HC, SQ], F8)       # [d-part, dc, q]
        ctxT = big.tile([128, HC, SQ], F8)     # [d-part, dc, q] = 32*ctx
        md = big.tile([128, KC, SQ], BF16)     # rel*(wdd1/32) (+am) [k-part, kc, q]
        wdd_b = big.tile([128, SQ], BF16)      # broadcast of wdd1/32
        wdd_row = big.tile([1, SQ], BF16)

        # ================= phase 1: projections + md =================
        with (
            tc.tile_pool(name="hs_pool", bufs=1) as hs_pool,
            tc.tile_pool(name="wpool", bufs=2) as wpool,
            tc.tile_pool(name="relp", bufs=3) as relp,
            tc.tile_pool(name="psum_p", bufs=4, space="PSUM") as psum_p,
        ):
            hsq_sb = hs_pool.tile([128, HC, SQ], F8)   # own rows (q==own keys)
            hso_sb = hs_pool.tile([128, HC, SQ], F8)   # other-half key rows
            hsqT_r = hsqT[:].rearrange("(c p) s -> p c s", p=128)
            hsoT_r = hsoT[:].rearrange("(c p) s -> p c s", p=128)

            wv_sb = wpool.tile([128, HC, H], F8, tag="w")
            wv_r = wvT[:].rearrange("(c p) m -> p c m", p=128)

            # DMA order: hsqT first (wdd + v first-half), wv, then hso.
            nc.sync.dma_start(hsq_sb[:, :, 0:512], hsqT_r[:, :, 0:512])
            nc.sync.dma_start(hsq_sb[:, :, 512:1024], hsqT_r[:, :, 512:1024])
            nc.sync.dma_start(wv_sb[:, :, 0:512], wv_r[:, :, 0:512])
            nc.sync.dma_start(wv_sb[:, :, 512:1024], wv_r[:, :, 512:1024])
            nc.sync.dma_start(hso_sb[:, :, 0:512], hsoT_r[:, :, 0:512])
            nc.sync.dma_start(hso_sb[:, :, 512:1024], hsoT_r[:, :, 512:1024])

            # rel tiles (f8 0/1 mask), one DMA per kc chunk of 128 keys
            def dma_rel(kc):
                t = relp.tile([128, SQ], F8, tag="rel")
                nc.sync.dma_start(t, relT[kc * 128:(kc + 1) * 128, :])
                return t

            # ---- wdd1/32 row: M=1 DR matmuls from hsqT ----
            for qn in range(2):
                q_sl = slice(qn * 512, (qn + 1) * 512)
                ps1 = psum_sm.tile([1, 512], F32, tag="small")
                for dc in range(0, HC, 2):
                    nc.tensor.matmul(
                        ps1,
                        u_t[:, dc:dc + 2, :],
                        hsq_sb[:, dc:dc + 2, q_sl],
                        start=(dc == 0),
                        stop=(dc == HC - 2),
                        perf_mode=DR,
                    )
                # wdd1/32 = psum/(64*32) + cq/32
                nc.vector.tensor_scalar(
                    wdd_row[:, q_sl], ps1, 1.0 / 2048.0, cq_t,
                    MULT, ADD,
                )
            nc.gpsimd.partition_broadcast(wdd_b, wdd_row)

            # md emission helper: one (kc) tile does both qn halves
            md_state = {"kc": 0}

            def emit_md_steps(n):
                for _ in range(n):
                    kc = md_state["kc"]
                    if kc >= KC:
                        return
                    md_state["kc"] += 1
                    rel_t = dma_rel(kc)
                    for qn in range(2):
                        q_sl = slice(qn * 512, (qn + 1) * 512)
                        nc.vector.tensor_mul(
                            md[:, kc, q_sl], rel_t[:, q_sl], wdd_b[:, q_sl]
                        )
                    if not am_zero:
                        nc.vector.tensor_scalar_add(
                            md[:, kc, :], md[:, kc, :], am_t[:, kc:kc + 1]
                        )

            # ---- v projection: v[s,d] for both halves ----
            # psum tile (half, sc, n0): lhsT = hs half chunk, rhs = WvT cols
            for half, src in ((0, hsq_sb), (1, hso_sb)):
                for n0 in range(0, H, 512):
                    for scl in range(KCH):
                        ps = psum_p.tile([128, 512], F32, tag="pp")
                        for dc in range(0, HC, 2):
                            nc.tensor.matmul(
                                ps,
                                src[:, dc:dc + 2, scl * 128:(scl + 1) * 128],
                                wv_sb[:, dc:dc + 2, n0:n0 + 512],
                                start=(dc == 0),
                                stop=(dc == HC - 2),
                                perf_mode=DR,
                            )
                        nc.vector.tensor_scalar_mul(
                            v_sb[:, half * KCH + scl, n0:n0 + 512], ps, 0.125
                        )
                        emit_md_steps(1 if (scl % 4 == 0) else 0)

            # ---- k projection -> kT[h, k] ; q projection -> qT[h, q] ----
            for w_d, bias_t, dst, srcs in (
                (wkT, bk_t, kT, ((0, hsq_sb), (1, hso_sb))),
                (wqT, bq_t, qT, ((0, hsq_sb),)),
            ):
                w_sb = wpool.tile([128, HC, H], F8, tag="w")
                w_r = w_d[:].rearrange("(c p) m -> p c m", p=128)
                for mc in range(HC):
                    nc.sync.dma_start(
                        w_sb[:, :, mc * 128:(mc + 1) * 128],
                        w_r[:, :, mc * 128:(mc + 1) * 128],
                    )
                for half, src in srcs:
                    for n0 in range(0, SQ, 512):
                        for mc in range(HC):
                            ps = psum_p.tile([128, 512], F32, tag="pp")
                            for dc in range(0, HC, 2):
                                nc.tensor.matmul(
                                    ps,
                                    w_sb[:, dc:dc + 2, mc * 128:(mc + 1) * 128],
                                    src[:, dc:dc + 2, n0:n0 + 512],
                                    start=(dc == 0),
                                    stop=(dc == HC - 2),
                                    perf_mode=DR,
                                )
                            col = half * SQ + n0
                            # evacuate on ACT: dst = psum*0.125 + bias
                            nc.scalar.activation(
                                dst[:, mc, col:col + 512], ps,
                                mybir.ActivationFunctionType.Identity,
                                bias=bias_t[:, mc:mc + 1],
                                scale=0.125,
                            )
                            emit_md_steps(1 if (mc % 4 == 0) else 0)
            emit_md_steps(KC)  # flush any remainder

        # ====== phase 2+3 interleaved per query-chunk of 512 ======
        with (
            tc.tile_pool(name="expp", bufs=2) as expp,
            tc.tile_pool(name="smx", bufs=3) as smx,
            tc.tile_pool(name="wo_pool", bufs=1) as wo_pool,
            tc.tile_pool(name="hsq_pool", bufs=1) as hsq_pool,
            tc.tile_pool(name="epi", bufs=3) as epi,
            tc.tile_pool(name="denp", bufs=2) as denp,
            tc.tile_pool(name="stat", bufs=4) as stat,
            tc.tile_pool(name="psum_s", bufs=3, space="PSUM") as psum_s,
            tc.tile_pool(name="psum_v", bufs=3, space="PSUM") as psum_v,
        ):
            wo_sb = wo_pool.tile([128, HC, H], F8)
            wo_r = woT[:].rearrange("(c p) m -> p c m", p=128)
            for dc in range(HC):
                nc.sync.dma_start(wo_sb[:, dc], wo_r[:, dc])
            hsq_sb2 = hsq_pool.tile([128, HC, H], F32)
            hsq_r = hsq[:].rearrange("(c p) m -> p c m", p=128)
            for sc in range(HC):
                nc.sync.dma_start(
                    hsq_sb2[:, sc], hsq_r[:, sc]
                )

            def scores_phase(qn):
                q_sl = slice(qn * 512, (qn + 1) * 512)
                expT = expp.tile([128, KC, 512], F8, tag="expT")
                dn = psum_sm.tile([1, 512], F32, tag="small")
                for kc in range(KC):
                    ps = psum_s.tile([128, 512], F32, tag="ps")
                    for dc in range(0, HC, 2):
                        nc.tensor.matmul(
                            ps,
                            kT[:, dc:dc + 2, kc * 128:(kc + 1) * 128],
                            qT[:, dc:dc + 2, q_sl],
                            start=(dc == 0),
                            stop=(dc == HC - 2),
                            perf_mode=DR,
                        )
                    y_t = smx.tile([128, 512], BF16, tag="y")
                    # y = ps/32 + md  (fused)
                    nc.vector.scalar_tensor_tensor(
                        y_t, ps, INV_SQRT_DH, md[:, kc, q_sl], MULT, ADD
                    )
                    nc.scalar.activation(
                        expT[:, kc, :], y_t, mybir.ActivationFunctionType.Exp
                    )
                # denominator: M=1 DR row-sums, lagged behind the exp chain
                for kc in range(0, KC, 2):
                    nc.tensor.matmul(
                        dn,
                        ones2,
                        expT[:, kc:kc + 2, :],
                        start=(kc == 0),
                        stop=(kc == KC - 2),
                        perf_mode=DR,
                    )
                dr = denp.tile([1, 512], F32, tag="dr")
                nc.vector.tensor_scalar_mul(dr, dn, INV_SQRT_DH)
                db = denp.tile([128, 512], F32, tag="db")
                nc.gpsimd.partition_broadcast(db, dr)
                rb = denp.tile([128, 512], F32, tag="rb")
                nc.vector.reciprocal(rb, db)  # rb = 32/denom
                return expT, rb

            def pv_phase(qn, expT, rb):
                q_sl = slice(qn * 512, (qn + 1) * 512)
                for dc in range(HC):
                    pv = psum_v.tile([128, 512], F32, tag="pv")
                    for kc in range(0, KC, 2):
                        nc.tensor.matmul(
                            pv,
                            v_sb[:, kc:kc + 2, dc * 128:(dc + 1) * 128],
                            expT[:, kc:kc + 2, :],
                            start=(kc == 0),
                            stop=(kc == KC - 2),
                            perf_mode=DR,
                        )
                    nc.vector.tensor_mul(ctxT[:, dc, q_sl], pv, rb)

            def epilogue(qn):
                # out-proj (fp8 DR) + residual + LN per 128-row chunk
                for sc in range(qn * 4, qn * 4 + 4):
                    x_t = epi.tile([128, H], F32, tag="x")
                    for hn in range(2):
                        ao = psum_v.tile([128, 512], F32, tag="pv")
                        for dc in range(0, HC, 2):
                            nc.tensor.matmul(
                                ao,
                                ctxT[:, dc:dc + 2, sc * 128:(sc + 1) * 128],
                                wo_sb[:, dc:dc + 2, hn * 512:(hn + 1) * 512],
                                start=(dc == 0),
                                stop=(dc == HC - 2),
                                perf_mode=DR,
                            )
                        h_sl = slice(hn * 512, (hn + 1) * 512)
                        # x = ao/256 + hsq  (fused; gpsimd has no PSUM port)
                        nc.vector.scalar_tensor_tensor(
                            x_t[:, h_sl], ao, 1.0 / 256.0,
                            hsq_sb2[:, sc, h_sl], MULT, ADD,
                        )

                    # LayerNorm over h (free dim)
                    st = stat.tile([128, 2, 6], F32, tag="st")
                    nc.vector.bn_stats(st[:, 0, :], x_t[:, 0:512])
                    nc.vector.bn_stats(st[:, 1, :], x_t[:, 512:1024])
                    mv = stat.tile([128, 2], F32, tag="mv")
                    nc.vector.bn_aggr(mv, st)
                    sd = stat.tile([128, 1], F32, tag="sd")
                    nc.scalar.activation(
                        sd, mv[:, 1:2],
                        mybir.ActivationFunctionType.Sqrt, bias=eps_t,
                    )
                    rq = stat.tile([128, 1], F32, tag="rq")
                    nc.vector.reciprocal(rq, sd)
                    nmur = stat.tile([128, 1], F32, tag="nm")
                    # nmur = -mu/sd
                    nc.vector.tensor_scalar(
                        nmur, mv[:, 0:1], rq, -1.0, MULT, MULT
                    )
                    y_t = epi.tile([128, H], F32, tag="yout")
                    nc.scalar.activation(
                        y_t, x_t, mybir.ActivationFunctionType.Identity,
                        bias=nmur, scale=rq,
                    )
                    if ln_affine:
                        nc.vector.tensor_mul(y_t, y_t, g_b)
                        nc.vector.tensor_add(y_t, y_t, b_b)
                    nc.sync.dma_start(out_d[sc * 128:(sc + 1) * 128, :], y_t)

            exp0, rb0 = scores_phase(0)
            pv_phase(0, exp0, rb0)
            exp1, rb1 = scores_phase(1)
            epilogue(0)
            pv_phase(1, exp1, rb1)
            epilogue(1)

    nc.compile()
    return nc


def get_program(am_zero=True, ln_affine=False):
    key = ("nc", am_zero, ln_affine)
    if key not in _CACHE:
        _CACHE[key] = _build_program(am_zero, ln_affine)
    return _CACHE[key]


def make_in_maps(inputs, am_zero=None, ln_affine=None):
    """Host-side sharding / layout prep (numpy only)."""
    f32 = np.float32
    f8 = ml_dtypes.float8_e4m3
    hs = np.asarray(inputs["hidden_states"], dtype=f32)
    rel = np.asarray(inputs["word_word_relation"])
    am = np.asarray(inputs["attention_mask"], dtype=f32)  # [B,1,1,S]
    Wq = np.asarray(inputs["Wq"], dtype=f32)
    Wk = np.asarray(inputs["Wk"], dtype=f32)
    Wv = np.asarray(inputs["Wv"], dtype=f32)
    Wo = np.asarray(inputs["Wo"], dtype=f32)
    bq = np.asarray(inputs["bq"], dtype=f32)
    bk = np.asarray(inputs["bk"], dtype=f32)
    bv = np.asarray(inputs["bv"], dtype=f32)
    bo = np.asarray(inputs["bo"], dtype=f32)
    d1 = np.asarray(inputs["dist_emb"], dtype=f32)[1]
    lng = np.asarray(inputs["ln_g"], dtype=f32)
    lnb = np.asarray(inputs["ln_b"], dtype=f32)
    if am_zero is None:
        am_zero = bool(np.all(am == 0.0))
    if ln_affine is None:
        ln_affine = not (np.all(lng == 1.0) and np.all(lnb == 0.0))

    wqT = np.ascontiguousarray(Wq.T * 8.0).astype(f8)
    wkT = np.ascontiguousarray(Wk.T * 8.0).astype(f8)
    wvT = np.ascontiguousarray(Wv.T * 8.0).astype(f8)
    woT = np.ascontiguousarray(Wo.T * 8.0).astype(f8)
    bo_eff = Wo @ bv + bo  # v/o biases fold into the residual
    bq_t = np.ascontiguousarray(bq.reshape(HC, 128).T)
    bk_t = np.ascontiguousarray(bk.reshape(HC, 128).T)
    u = (Wq.astype(np.float64).T @ d1.astype(np.float64)).astype(f32)
    u_t = np.ascontiguousarray((u * 64.0).reshape(HC, 128).T)[:, :, None].astype(f8)
    cq32 = np.array([[float(bq @ d1) / 32.0]], dtype=f32)
    relm = (rel == 1)

    in_maps = []
    for c in range(N_CORES):
        b, qh = divmod(c, 2)
        qs = qh * SQ
        os_ = (1 - qh) * SQ
        kidx = np.r_[qs:qs + SQ, os_:os_ + SQ]   # [own keys, other keys]
        m = {
            "hsqT": np.ascontiguousarray(hs[b, qs:qs + SQ, :].T).astype(f8),
            "hsoT": np.ascontiguousarray(hs[b, os_:os_ + SQ, :].T).astype(f8),
            "hsq": hs[b, qs:qs + SQ, :] + bo_eff,
            "relT": np.ascontiguousarray(
                relm[b, qs:qs + SQ, :].T[kidx, :]).astype(f8),
            "wqT": wqT, "wkT": wkT, "wvT": wvT, "woT": woT,
            "u": u_t, "cq32": cq32, "bq": bq_t, "bk": bk_t,
        }
        if not am_zero:
            m["am"] = np.ascontiguousarray(
                am[b, 0, 0][kidx].reshape(KC, 128).T)
        if ln_affine:
            m["lng"] = lng
            m["lnb"] = lnb
        in_maps.append(m)
    return in_maps


def kernel(**inputs):
    am = np.asarray(inputs["attention_mask"], dtype=np.float32)
    am_zero = bool(np.all(am == 0.0))
    ln_affine = not (
        np.all(np.asarray(inputs["ln_g"]) == 1.0)
        and np.all(np.asarray(inputs["ln_b"]) == 0.0)
    )
    nc = get_program(am_zero, ln_affine)
    in_maps = make_in_maps(inputs, am_zero, ln_affine)
    res = run_bass_kernel_spmd(nc, in_maps, core_ids=list(range(N_CORES)))
    out = np.empty((B, S, H), dtype=np.float32)
    for c in range(N_CORES):
        b, qh = divmod(c, 2)
        out[b, qh * SQ:(qh + 1) * SQ, :] = res.results[c]["out"]
    return out
